# revision 13
# baseline (speedup 1.0000x reference)
"""GQA attention (RoPE, no mask) sharded over 8 NeuronCores.

Sharding: TP over the 4 KV-head groups x DP over batch (2).
core c -> batch b = c//4, kv-group g = c%4 (query heads 4g..4g+3).
Each core computes Q/K/V projections for its heads, RoPE, softmax(QK^T)V,
and its o_proj partial; the 4 partials per batch are summed host-side.

Key structure (v2):
- Scores ST = [s, q] (K-major); exp runs as one 1024-wide ACT op per
  head-pair covering both heads' PSUM banks.
- PV is computed transposed: out[q, h] with lhsT = exp-probs slices and
  rhs = V extended with a ones column, so the softmax denominator lands
  in column H as a per-partition scalar. Normalization is then a cheap
  [128,1] reciprocal + per-partition-scale copy, transposed back to
  [h, q] on the PE for o_proj. Two accumulation groups share each PSUM
  bank: only the first matmul into a bank uses start=True (bank-wide
  has_written clear); the second group's first matmul relies on
  overwrite-on-cleared-bit.
- RoPE: 4 DVE ops per 512-chunk using cos/sin tables duplicated across
  both partition halves.
- DMA: x chunks split into 4 sub-DMAs on the sync queue (projection
  matmuls start as soon as the first quarter lands); weights/tables on
  the scalar queue in first-use order; output stored bf16.
"""

import sys

sys.path.insert(0, "/opt/trn_rl_repo")

from contextlib import ExitStack

import ml_dtypes
import numpy as np

import concourse.bass as bass
import concourse.tile as tile
from concourse import bacc, mybir
from concourse.bass_utils import run_bass_kernel_spmd

BF16 = mybir.dt.bfloat16
F32 = mybir.dt.float32
NP_BF16 = ml_dtypes.bfloat16

B, T_FULL, S_FULL, D_FULL = 2, 2048, 2048, 2048
N_HEADS, KV_HEADS, H = 16, 4, 128
HG = N_HEADS // KV_HEADS  # query heads per core (4)
HD = HG * H  # per-core q head dims (512)
MIN_TS, MAX_TS = 1.0, 10000.0


def build(T=T_FULL, S=S_FULL, D=D_FULL, repeat=1):
    """Build the per-core Bass graph. Returns compiled nc."""
    assert T % 512 == 0 and S % 512 == 0 and D % 128 == 0
    TQC = T // 512  # q chunks of 512
    SC = S // 512  # s chunks of 512
    S128 = S // 128  # s chunks of 128
    DC = D // 128  # contraction chunks of 128

    nc = bacc.Bacc("TRN2", target_bir_lowering=False, debug=False, num_devices=8)

    id_d = nc.dram_tensor("ident", [128, 128], BF16, kind="ExternalInput").ap()
    xq_d = nc.dram_tensor("XqT", [T // 512, 128, DC, 512], BF16, kind="ExternalInput").ap()
    xkv_d = nc.dram_tensor("XkvT", [S // 512, 128, DC, 512], BF16, kind="ExternalInput").ap()
    wq_d = nc.dram_tensor("Wq", [128, DC, HD], BF16, kind="ExternalInput").ap()
    wk_d = nc.dram_tensor("Wk", [128, DC, H], BF16, kind="ExternalInput").ap()
    wv_d = nc.dram_tensor("Wv", [128, DC, H], BF16, kind="ExternalInput").ap()
    wo_d = nc.dram_tensor("Wo", [128, HG, D], BF16, kind="ExternalInput").ap()
    cosq_d = nc.dram_tensor("cos_q", [H // 2, T], F32, kind="ExternalInput").ap()
    sinq_d = nc.dram_tensor("sin_q", [H // 2, T], F32, kind="ExternalInput").ap()
    cosk_d = nc.dram_tensor("cos_k", [H // 2, S], F32, kind="ExternalInput").ap()
    sink_d = nc.dram_tensor("sin_k", [H // 2, S], F32, kind="ExternalInput").ap()
    out_d = nc.dram_tensor("out", [T, D], BF16, kind="ExternalOutput").ap()

    with tile.TileContext(nc) as tc, ExitStack() as ctx:
        wpool = ctx.enter_context(tc.tile_pool(name="w", bufs=1))
        xpool = ctx.enter_context(tc.tile_pool(name="x", bufs=3))
        qkv = ctx.enter_context(tc.tile_pool(name="qkv", bufs=1))
        ptp = ctx.enter_context(tc.tile_pool(name="pt", bufs=4))
        tmpp = ctx.enter_context(tc.tile_pool(name="tmp", bufs=2))
        recp = ctx.enter_context(tc.tile_pool(name="rec", bufs=8))
        aqp = ctx.enter_context(tc.tile_pool(name="aq", bufs=4))
        outp = ctx.enter_context(tc.tile_pool(name="outs", bufs=2))
        # PSUM: "st" slots are [128,1024] f32 (2 banks each) shared by score
        # pairs, projections, o_proj, and attn transposes; "pv" slots hold
        # two [128,129] accumulation groups per bank.
        ps_st = ctx.enter_context(tc.tile_pool(name="ps_st", bufs=2, space="PSUM"))
        ps_pv = ctx.enter_context(tc.tile_pool(name="ps_pv", bufs=4, space="PSUM"))

        # ---- weights / tables on the gpsimd DMA queue in first-use order
        # (sync + scalar queues are reserved for the x input stream).
        # cos/sin tables are DMA'd once into rows 0:64 and duplicated into
        # rows 64:128 on-chip so rope can run [128,512] DVE ops.
        wk_sb = wpool.tile([128, DC, H], BF16, tag="wk")
        nc.gpsimd.dma_start(wk_sb[:], wk_d[:])
        ident = wpool.tile([128, 128], BF16, tag="ident")
        nc.gpsimd.dma_start(ident[:], id_d[:])
        cck = wpool.tile([128, S], F32, tag="cck")
        ssk = wpool.tile([128, S], F32, tag="ssk")
        nc.gpsimd.dma_start(cck[0:64, :], cosk_d[:])
        nc.gpsimd.dma_start(ssk[0:64, :], sink_d[:])
        wv_sb = wpool.tile([128, DC, H], BF16, tag="wv")
        nc.gpsimd.dma_start(wv_sb[:], wv_d[:])
        wq_sb = wpool.tile([128, DC, HD], BF16, tag="wq")
        nc.gpsimd.dma_start(wq_sb[:], wq_d[:])
        ccq = wpool.tile([128, T], F32, tag="ccq")
        ssq = wpool.tile([128, T], F32, tag="ssq")
        nc.gpsimd.dma_start(ccq[0:64, :], cosq_d[:])
        nc.gpsimd.dma_start(ssq[0:64, :], sinq_d[:])
        wo_sb = wpool.tile([128, HG, D], BF16, tag="wo")
        nc.gpsimd.dma_start(wo_sb[:], wo_d[:])
        for tbl in (cck, ssk, ccq, ssq):
            nc.vector.tensor_copy(tbl[64:128, :], tbl[0:64, :])

        qt_sb = qkv.tile([128, HG, T], BF16, tag="qt")
        kt_sb = qkv.tile([128, S], BF16, tag="kt")
        vt_sb = qkv.tile([128, S], BF16, tag="vt")
        v_ext = qkv.tile([128, S128, H + 1], BF16, tag="vx")
        ot_sb = qkv.tile([128, HG, T], BF16, tag="ot")

        def load_x(src_chunk):
            # 4 sub-DMAs: projection matmuls start on the first quarter and
            # each chunk's transfer pipelines with compute.
            x = xpool.tile([128, DC, 512], BF16, tag="x")
            for i in range(4):
                nc.sync.dma_start(x[:, 4 * i : 4 * i + 4, :], src_chunk[:, 4 * i : 4 * i + 4, :])
            return x

        def rope(dst, psq, cc_ap, ss_ap):
            # psq: [128, 1024] PSUM tile; cols 0:512 hold the projection,
            # cols 512:1024 (the slot's second bank) are scratch for the
            # sin-product so the partition-shifted combine operand is PSUM
            # (walrus requires equal base partitions when both DVE inputs
            # are SBUF).
            # dst[0:64] = ps[0:64]*cos - ps[64:128]*sin
            # dst[64:128] = ps[64:128]*cos + ps[0:64]*sin
            t0 = tmpp.tile([128, 512], F32, tag="t0")
            nc.vector.tensor_mul(t0[:], psq[:, 0:512], cc_ap)
            nc.vector.tensor_mul(psq[:, 512:1024], psq[:, 0:512], ss_ap)
            nc.vector.tensor_sub(dst[0:64, :], t0[0:64, :], psq[64:128, 512:1024])
            nc.vector.tensor_add(dst[64:128, :], t0[64:128, :], psq[0:64, 512:1024])

        def body():
            nc.vector.memset(v_ext[:, :, H : H + 1], 1.0)

            def emit_transposes(j):
                # V[s, h] = transpose of VT[h, s] per 128x128 block
                for sub in range(4):
                    trt = ps_pv.tile([128, 1024], BF16, tag="pv", name=f"vtr{j}_{sub}")
                    nc.tensor.transpose(
                        trt[:, 0:128], vt_sb[:, bass.ts(4 * j + sub, 128)], ident[:]
                    )
                    nc.vector.tensor_copy(v_ext[:, 4 * j + sub, 0:H], trt[:, 0:128])

            # ---- K/V projections ----
            for j in range(SC):
                xk = load_x(xkv_d[j])
                psk = ps_st.tile([128, 1024], F32, tag="st", name=f"psk{j}")
                for d in range(DC):
                    nc.tensor.matmul(
                        psk[:, 0:512], wk_sb[:, d, :], xk[:, d, :],
                        start=(d == 0), stop=(d == DC - 1),
                    )
                rope(
                    kt_sb[:, bass.ts(j, 512)], psk,
                    cck[:, bass.ts(j, 512)], ssk[:, bass.ts(j, 512)],
                )
                psv = ps_st.tile([128, 512], F32, tag="st", name=f"psv{j}")
                for d in range(DC):
                    nc.tensor.matmul(
                        psv[:], wv_sb[:, d, :], xk[:, d, :],
                        start=(d == 0), stop=(d == DC - 1),
                    )
                nc.scalar.copy(vt_sb[:, bass.ts(j, 512)], psv[:])
                if j > 0:
                    emit_transposes(j - 1)
            emit_transposes(SC - 1)

            # ---- per q-chunk attention ----
            def qproj(qc):
                xq = load_x(xq_d[qc])
                for hh in range(HG):
                    psq = ps_st.tile([128, 1024], F32, tag="st", name=f"psq{qc}_{hh}")
                    for d in range(DC):
                        nc.tensor.matmul(
                            psq[:, 0:512], wq_sb[:, d, bass.ts(hh, 128)], xq[:, d, :],
                            start=(d == 0), stop=(d == DC - 1),
                        )
                    rope(
                        qt_sb[:, hh, bass.ts(qc, 512)], psq,
                        ccq[:, bass.ts(qc, 512)], ssq[:, bass.ts(qc, 512)],
                    )

            def normalize_chain(bank, sub, hh, qc, j, trt):
                # bank[:, sub, :] = [q128, H+1] unnormalized attn + denom col.
                rec = recp.tile([128, 1], F32, tag="rec", name=f"rec{qc}_{hh}_{j}")
                nc.vector.reciprocal(rec[:], bank[:, sub, H : H + 1])
                aqh = aqp.tile([128, 128], BF16, tag="aqh", name=f"aqh{qc}_{hh}_{j}")
                nc.scalar.mul(aqh[:], bank[:, sub, 0:H], rec[:])
                off = 128 * (2 * (j % 2))  # transposes pair into one trt slot
                nc.tensor.transpose(trt[:, off : off + 128], aqh[:], ident[:])
                nc.vector.tensor_copy(
                    ot_sb[:, hh, qc * 512 + j * 128 : qc * 512 + (j + 1) * 128],
                    trt[:, off : off + 128],
                )

            def drain_chains(pending, n):
                emitted = 0
                trt = None
                while pending and emitted < n:
                    if emitted % 2 == 0:
                        trt = ps_pv.tile(
                            [128, 1024], BF16, tag="pv",
                            name=f"atr{len(pending)}_{emitted}",
                        )
                    normalize_chain(*pending.pop(0), trt)
                    emitted += 1

            qproj(0)
            pending = []
            for qc in range(TQC):
                for hp in range(HG // 2):
                    h0, h1 = 2 * hp, 2 * hp + 1
                    st_tiles = [None] * S128

                    def emit_st(s):
                        pst = ps_st.tile([128, 1024], F32, tag="st", name=f"pst{s}")
                        nc.tensor.matmul(
                            pst[:, 0:512], kt_sb[:, bass.ts(s, 128)],
                            qt_sb[:, h0, bass.ts(qc, 512)], start=True, stop=True,
                        )
                        nc.tensor.matmul(
                            pst[:, 512:1024], kt_sb[:, bass.ts(s, 128)],
                            qt_sb[:, h1, bass.ts(qc, 512)], start=True, stop=True,
                        )
                        st_tiles[s] = pst

                    # two lookahead score pairs give the PE runway while the
                    # previous hp's normalize chains drain
                    emit_st(0)
                    emit_st(1)
                    drain_chains(pending, 99)
                    # PV accumulators: 2 banks per head, 2 q-sub groups per
                    # bank. Allocated AFTER the drain so the pool rotation
                    # (trt tiles reuse the previous hp's pv slots) stays
                    # deadlock-free.
                    pvb = [
                        ps_pv.tile(
                            [128, 2, 129], F32, tag="pv",
                            padded_shape=[128, 2, 256], name=f"pv{qc}_{hp}_{i}",
                        )
                        for i in range(4)
                    ]
                    for s in range(S128):
                        if s + 2 < S128:
                            emit_st(s + 2)
                        pst = st_tiles[s]
                        st_tiles[s] = None
                        pt = ptp.tile([128, 1024], BF16, tag="pt", name=f"pt{s}")
                        nc.scalar.activation(
                            pt[:], pst[:], mybir.ActivationFunctionType.Exp
                        )
                        for hx, off in ((0, 0), (1, 512)):
                            for j in range(4):
                                bank = pvb[2 * hx + j // 2]
                                nc.tensor.matmul(
                                    bank[:, j % 2, :],
                                    pt[:, off + 128 * j : off + 128 * (j + 1)],
                                    v_ext[:, s, :],
                                    start=(s == 0 and j % 2 == 0),
                                    stop=(s == S128 - 1),
                                    skip_group_check=True,
                                )
                    for hx, hh in ((0, h0), (1, h1)):
                        for j in range(4):
                            pending.append((pvb[2 * hx + j // 2], j % 2, hh, qc, j))

                if qc + 1 < TQC:
                    qproj(qc + 1)
                drain_chains(pending, 99)

                # ---- o_proj for this q chunk ----
                for tsub in range(4):
                    trow = qc * 512 + tsub * 128
                    ostage = outp.tile([128, D], BF16, tag="ostage", name=f"os{tsub}")
                    for dp in range(D // 1024):
                        pso2 = ps_st.tile([128, 1024], F32, tag="st", name=f"pso{tsub}_{dp}")
                        for half in range(2):
                            for hh in range(HG):
                                nc.tensor.matmul(
                                    pso2[:, 512 * half : 512 * (half + 1)],
                                    ot_sb[:, hh, trow : trow + 128],
                                    wo_sb[:, hh, bass.ts(2 * dp + half, 512)],
                                    start=(hh == 0), stop=(hh == HG - 1),
                                )
                        nc.vector.tensor_copy(ostage[:, bass.ts(dp, 1024)], pso2[:])
                    nc.scalar.dma_start(out_d[trow : trow + 128, :], ostage[:])

        if repeat == 1:
            body()
        else:
            with tc.For_i(0, repeat):
                body()

    nc.compile()
    return nc


def _shard_inputs(Xq, Xkv, q_positions, kv_positions, Wq, Wk, Wv, Wo):
    """Build per-core input maps. Core c: batch c//4, kv-group c%4."""
    D = Xq.shape[2]
    half = H // 2
    frac = 2.0 * np.arange(half, dtype=np.float32) / H
    ts = (MIN_TS * (MAX_TS / MIN_TS) ** frac).astype(np.float32)

    def tables(pos):
        s = pos.astype(np.float32)[None, :] / ts[:, None]
        return np.cos(s).astype(np.float32), np.sin(s).astype(np.float32)

    DC = D // 128

    def chunked_xT(X):
        # [L, D] -> X.T laid out as [L//512, 128, DC, 512]: contiguous per partition
        xt = np.ascontiguousarray(X.T).astype(NP_BF16)  # [D, L]
        L = X.shape[0]
        return np.ascontiguousarray(
            xt.reshape(DC, 128, L // 512, 512).transpose(2, 1, 0, 3)
        )

    def chunked_w(W, m):
        # [D, m] -> [128, DC, m]
        return np.ascontiguousarray(
            W.reshape(DC, 128, m).transpose(1, 0, 2)
        ).astype(NP_BF16)

    in_maps = []
    for c in range(8):
        b, g = c // 4, c % 4
        cq, sq = tables(q_positions[b])
        ck, sk = tables(kv_positions[b])
        in_maps.append(
            {
                "XqT": chunked_xT(Xq[b]),
                "XkvT": chunked_xT(Xkv[b]),
                "Wq": chunked_w(Wq[:, HG * g : HG * (g + 1), :].reshape(D, HD), HD),
                "Wk": chunked_w(Wk[:, g, :], H),
                "Wv": chunked_w(Wv[:, g, :], H),
                # Wo [HG, 128, D] -> [128, HG, D]: wo_sb[h, hh, d] = Wo[g*HG+hh, h, d]
                "Wo": np.ascontiguousarray(
                    Wo[HG * g : HG * (g + 1)].transpose(1, 0, 2)
                ).astype(NP_BF16),
                "cos_q": cq, "sin_q": sq, "cos_k": ck, "sin_k": sk,
                "ident": np.eye(128, dtype=NP_BF16),
            }
        )
    return in_maps


_NC_CACHE = {}


def kernel(Xq, Xkv, q_positions, kv_positions, Wq, Wk, Wv, Wo):
    key = ("full", 1)
    if key not in _NC_CACHE:
        _NC_CACHE[key] = build()
    nc = _NC_CACHE[key]
    in_maps = _shard_inputs(Xq, Xkv, q_positions, kv_positions, Wq, Wk, Wv, Wo)
    res = run_bass_kernel_spmd(nc, in_maps, core_ids=list(range(8)))
    T, D = Xq.shape[1], Xq.shape[2]
    out = np.zeros((B, T, D), dtype=np.float32)
    for c in range(8):
        out[c // 4] += res.results[c]["out"].astype(np.float32)
    return out
